# revision 3
# baseline (speedup 1.0000x reference)
"""Trainium2 Bass kernel for DLiNOSSLayer (nn_DLiNOSSLayer_85976655331766).

Math: per (b,h,s) lane the layer is the linear recurrence
    h[t] = a[s] * h[t-1] + beta[t],   a = r * exp(i*omega),  r real in (0,1]
Rotating-frame trick: with h[t] = exp(i*omega*(t+1)) * v[t] the coefficient
becomes the real r, so the complex scan splits into two independent real
scans that map 1:1 onto the DVE's native tensor_tensor_scan instruction
(state = data0*state + data1).  Twiddle tables exp(-i*omega*(t+1)) are
precomputed on host in float64.

Sharding: batch (4) x head-halves (2) -> 8 cores, no cross-core comm.
Per core: 8 heads as 4 head-pairs packed into 128 partitions.
"""

import numpy as np

H, DH, S = 16, 64, 64
S_FAST = 48
BATCH, L, D = 4, 2048, 1024
N_CORES = 8
PAIRS = 4          # head-pairs per core (8 heads)
HALF = L // 2      # free-dim tile length

_CACHE = {}


def _np_softplus(x):
    return np.logaddexp(0.0, x)


def _np_sigmoid(x):
    return 1.0 / (1.0 + np.exp(-x))


# ---------------------------------------------------------------------------
# general fallback (pure numpy, matches reference semantics for any inputs)
# ---------------------------------------------------------------------------
def _fallback(x, log_alpha, omega, B_re, B_im, C_re, C_im, D_skip, W_g, b_g, h0):
    Bn, Ln, Dm = x.shape
    Hn, Stot = omega.shape
    Sf = log_alpha.shape[1]
    dh = Dm // Hn
    mag_fast = np.minimum(np.exp(-_np_softplus(log_alpha.astype(np.float32))), 0.999)
    mag = np.concatenate([mag_fast, np.ones((Hn, Stot - Sf), np.float32)], axis=1)
    A = (mag * np.exp(1j * omega.astype(np.float32))).astype(np.complex64)
    Bc = (B_re + 1j * B_im).astype(np.complex64)
    Cc = (C_re + 1j * C_im).astype(np.complex64)
    xr = x.reshape(Bn, Ln, Hn, dh)
    g = _np_sigmoid(np.einsum('hsd,blhd->blhs', W_g, xr) + b_g).astype(np.float32)
    Bx = np.einsum('hsd,blhd->blhs', Bc, xr.astype(np.complex64))
    alpha = np.empty((Bn, Ln, Hn, Stot), np.complex64)
    alpha[..., :Sf] = g[..., :Sf] * A[:, :Sf]
    alpha[..., Sf:] = A[:, Sf:]
    beta = (1.0 - g).astype(np.complex64) * Bx
    h = h0.astype(np.complex64).copy()
    h_all = np.empty((Bn, Ln, Hn, Stot), np.complex64)
    for t in range(Ln):
        h = alpha[:, t] * h + beta[:, t]
        h_all[:, t] = h
    y_c = np.einsum('hds,blhs->blhd', Cc, h_all)
    y = y_c.real.reshape(Bn, Ln, Dm).astype(np.float32) + x * D_skip.astype(np.float32)
    return y, h_all[:, -1]


# ---------------------------------------------------------------------------
# host-side parameter prep (float64)
# ---------------------------------------------------------------------------
def _prep_params(log_alpha, omega, b_g):
    la = np.asarray(log_alpha, np.float64)
    om = np.asarray(omega, np.float64)
    mag_fast = np.minimum(np.exp(-_np_softplus(la)), 0.999)
    mag = np.concatenate([mag_fast, np.ones((H, S - S_FAST))], axis=1)
    g = _np_sigmoid(np.asarray(b_g, np.float64))
    r = np.concatenate(
        [g[:, :S_FAST] * mag[:, :S_FAST], np.ones((H, S - S_FAST))], axis=1)
    one_minus_g = 1.0 - g
    return om, r, one_minus_g


def _build_core_inputs(ci, x, om, r, omg, B_re, C_re, D_skip, h0):
    """Per-core input arrays for core ci."""
    b, hg = ci // 2, ci % 2
    heads = slice(hg * 8, hg * 8 + 8)
    cols = slice(hg * 512, hg * 512 + 512)

    xst = np.ascontiguousarray(x[b].T[cols]).astype(np.float32)      # (512, 2048)

    om_c = om[heads].reshape(512)                                     # rows: (h,s)
    t_idx = np.arange(1, L + 1, dtype=np.float64)
    phase = om_c[:, None] * t_idx[None, :]                            # (512, L)
    tre = np.cos(phase).astype(np.float32)
    tim = (-np.sin(phase)).astype(np.float32)

    # weights (128, 1536): [wB p0..3 | wC p0..3 | wD p0..3]
    wts = np.zeros((128, 1536), np.float32)
    for p in range(PAIRS):
        h0i = hg * 8 + 2 * p
        for k in range(2):
            rs = slice(k * 64, k * 64 + 64)
            # gated Bx: out[s] = (1-g)[s] * sum_d B_re[h][s,d] x[d]
            # -> lhsT[d,s] = (diag(1-g) B_re[h]).T
            bg = (B_re[h0i + k].astype(np.float64)
                  * omg[h0i + k][:, None]).astype(np.float32)
            wts[rs, p * 128 + k * 64:p * 128 + k * 64 + 64] = bg.T
            # y: out[d] = sum_s C_re[h][d,s] h[s] -> lhsT[s,d] = C_re[h].T
            wts[rs, 512 + p * 128 + k * 64:512 + p * 128 + k * 64 + 64] = \
                C_re[h0i + k].T.astype(np.float32)
        gcols = hg * 512 + p * 128
        wts[:, 1024 + p * 128:1024 + p * 128 + 128] = \
            np.diag(D_skip[gcols:gcols + 128].astype(np.float32))

    # smalls (128, 16): col = kind*4 + p; kinds: r, (1-g), h0re, h0im
    smalls = np.zeros((128, 16), np.float32)
    r_c = r[heads].reshape(512)
    omg_c = omg[heads].reshape(512)
    h0_c = h0[b][heads].reshape(512)
    for p in range(PAIRS):
        rows = slice(p * 128, p * 128 + 128)
        smalls[:, 0 + p] = r_c[rows]
        smalls[:, 4 + p] = omg_c[rows]
        smalls[:, 8 + p] = np.ascontiguousarray(h0_c[rows].real).astype(np.float32)
        smalls[:, 12 + p] = np.ascontiguousarray(h0_c[rows].imag).astype(np.float32)

    return {"xst": xst, "tre": tre, "tim": tim, "wts": wts, "smalls": smalls}


# ---------------------------------------------------------------------------
# bass program
# ---------------------------------------------------------------------------
def _build_program():
    from contextlib import ExitStack
    import concourse.bass as bass
    import concourse.tile as tile
    from concourse import bacc, mybir

    f32 = mybir.dt.float32
    MUL = mybir.AluOpType.mult
    ADD = mybir.AluOpType.add
    IDENT = mybir.ActivationFunctionType.Identity

    nc = bacc.Bacc("TRN2", debug=False, num_devices=N_CORES)

    xst_d = nc.dram_tensor("xst", (512, 2048), f32, kind="ExternalInput").ap()
    tre_d = nc.dram_tensor("tre", (512, 2048), f32, kind="ExternalInput").ap()
    tim_d = nc.dram_tensor("tim", (512, 2048), f32, kind="ExternalInput").ap()
    wts_d = nc.dram_tensor("wts", (128, 1536), f32, kind="ExternalInput").ap()
    smalls_d = nc.dram_tensor("smalls", (128, 16), f32, kind="ExternalInput").ap()
    yst_d = nc.dram_tensor("yst", (512, 2048), f32, kind="ExternalOutput").ap()
    vlast_d = nc.dram_tensor("vlast", (4, 128, 2), f32, kind="ExternalOutput").ap()

    with ExitStack() as ctx:
        tc = ctx.enter_context(tile.TileContext(nc))
        const = ctx.enter_context(tc.tile_pool(name="const", bufs=1))
        rp = ctx.enter_context(tc.tile_pool(name="rmat", bufs=4))
        xp = ctx.enter_context(tc.tile_pool(name="xin", bufs=3))
        tp = ctx.enter_context(tc.tile_pool(name="twid", bufs=3))
        sp = ctx.enter_context(tc.tile_pool(name="work", bufs=2))
        vp = ctx.enter_context(tc.tile_pool(name="vscan", bufs=3))
        yp = ctx.enter_context(tc.tile_pool(name="yout", bufs=3))
        psA = ctx.enter_context(tc.tile_pool(name="psA", bufs=2, space="PSUM"))
        psB = ctx.enter_context(tc.tile_pool(name="psB", bufs=2, space="PSUM"))

        wts_t = const.tile([128, 1536], f32, tag="wts")
        nc.sync.dma_start(wts_t[:], wts_d[:, :])
        smalls_t = const.tile([128, 16], f32, tag="smalls")
        nc.sync.dma_start(smalls_t[:], smalls_d[:, :])
        scratch = const.tile([128, HALF], f32, tag="scratch")
        nc.vector.memset(scratch[:], 0.0)

        r_tiles = []
        for p in range(PAIRS):
            rt = rp.tile([128, HALF], f32, tag=f"r{p}")
            nc.scalar.activation(rt[:], scratch[:], IDENT,
                                 bias=smalls_t[:, p:p + 1], scale=0.0)
            r_tiles.append(rt)

        for p in range(PAIRS):
            rows = slice(p * 128, (p + 1) * 128)
            vre_prev = vim_prev = None
            for hf in range(2):
                tslc = slice(hf * HALF, (hf + 1) * HALF)
                x_t = xp.tile([128, HALF], f32, tag="x")
                nc.sync.dma_start(x_t[:], xst_d[rows, tslc])
                tre_t = tp.tile([128, HALF], f32, tag="tre")
                nc.sync.dma_start(tre_t[:], tre_d[rows, tslc])
                tim_t = tp.tile([128, HALF], f32, tag="tim")
                nc.sync.dma_start(tim_t[:], tim_d[rows, tslc])

                bx_ps = psA.tile([128, HALF], f32, tag="bxps")
                for j in range(2):
                    js = slice(j * 512, (j + 1) * 512)
                    nc.tensor.matmul(bx_ps[:, js],
                                     lhsT=wts_t[:, p * 128:(p + 1) * 128],
                                     rhs=x_t[:, js], start=True, stop=True)
                bx_sb = sp.tile([128, HALF], f32, tag="bxsb")
                nc.scalar.copy(bx_sb[:], bx_ps[:])

                phi_re = sp.tile([128, HALF], f32, tag="phre")
                nc.vector.tensor_tensor(phi_re[:], tre_t[:], bx_sb[:], op=MUL)
                phi_im = sp.tile([128, HALF], f32, tag="phim")
                nc.gpsimd.tensor_tensor(phi_im[:], tim_t[:], bx_sb[:], op=MUL)

                if hf == 0:
                    init_re = smalls_t[:, 8 + p:9 + p]
                    init_im = smalls_t[:, 12 + p:13 + p]
                else:
                    init_re = vre_prev[:, HALF - 1:HALF]
                    init_im = vim_prev[:, HALF - 1:HALF]
                v_re = vp.tile([128, HALF], f32, tag="vre")
                nc.vector.tensor_tensor_scan(
                    v_re[:], data0=r_tiles[p][:], data1=phi_re[:],
                    initial=init_re, op0=MUL, op1=ADD)
                v_im = vp.tile([128, HALF], f32, tag="vim")
                nc.vector.tensor_tensor_scan(
                    v_im[:], data0=r_tiles[p][:], data1=phi_im[:],
                    initial=init_im, op0=MUL, op1=ADD)

                p_t = sp.tile([128, HALF], f32, tag="pt")
                nc.gpsimd.tensor_tensor(p_t[:], tre_t[:], v_re[:], op=MUL)
                q_t = sp.tile([128, HALF], f32, tag="qt")
                nc.vector.tensor_tensor(q_t[:], tim_t[:], v_im[:], op=MUL)
                hre_t = sp.tile([128, HALF], f32, tag="hre")
                nc.vector.tensor_tensor(hre_t[:], p_t[:], q_t[:], op=ADD)

                y_ps = psB.tile([128, HALF], f32, tag="yps")
                for j in range(2):
                    js = slice(j * 512, (j + 1) * 512)
                    nc.tensor.matmul(y_ps[:, js],
                                     lhsT=wts_t[:, 512 + p * 128:512 + (p + 1) * 128],
                                     rhs=hre_t[:, js], start=True, stop=False)
                    nc.tensor.matmul(y_ps[:, js],
                                     lhsT=wts_t[:, 1024 + p * 128:1024 + (p + 1) * 128],
                                     rhs=x_t[:, js], start=False, stop=True)
                y_sb = yp.tile([128, HALF], f32, tag="ysb")
                nc.scalar.copy(y_sb[:], y_ps[:])
                nc.sync.dma_start(yst_d[rows, tslc], y_sb[:])

                if hf == 1:
                    nc.sync.dma_start(vlast_d[p, :, 0:1], v_re[:, HALF - 1:HALF])
                    nc.sync.dma_start(vlast_d[p, :, 1:2], v_im[:, HALF - 1:HALF])
                vre_prev, vim_prev = v_re, v_im

    nc.compile()
    return nc


def _get_program():
    if "nc" not in _CACHE:
        _CACHE["nc"] = _build_program()
    return _CACHE["nc"]


# ---------------------------------------------------------------------------
# entry point
# ---------------------------------------------------------------------------
def kernel(x, log_alpha, omega, B_re, B_im, C_re, C_im, D_skip, W_g, b_g, h0,
           _trace=False):
    x = np.asarray(x)
    if (np.any(np.asarray(W_g)) or np.any(np.asarray(B_im))
            or np.any(np.asarray(C_im))
            or x.shape != (BATCH, L, D) or omega.shape != (H, S)
            or log_alpha.shape != (H, S_FAST)):
        return _fallback(np.asarray(x), np.asarray(log_alpha), np.asarray(omega),
                         np.asarray(B_re), np.asarray(B_im), np.asarray(C_re),
                         np.asarray(C_im), np.asarray(D_skip), np.asarray(W_g),
                         np.asarray(b_g), np.asarray(h0))

    om, r, omg = _prep_params(log_alpha, omega, b_g)
    B_re = np.asarray(B_re)
    C_re = np.asarray(C_re)
    D_skip = np.asarray(D_skip)
    h0 = np.asarray(h0)

    in_maps = [
        _build_core_inputs(ci, x, om, r, omg, B_re, C_re, D_skip, h0)
        for ci in range(N_CORES)
    ]

    nc = _get_program()
    from concourse import bass_utils
    res = bass_utils.run_bass_kernel_spmd(
        nc, in_maps, core_ids=list(range(N_CORES)), trace=_trace)

    y = np.empty((BATCH, L, D), np.float32)
    h_last = np.empty((BATCH, H, S), np.complex64)
    for ci in range(N_CORES):
        b, hg = ci // 2, ci % 2
        out = res.results[ci]
        y[b][:, hg * 512:hg * 512 + 512] = out["yst"].T
        vl = out["vlast"]                                   # (4,128,2)
        v = (vl[..., 0] + 1j * vl[..., 1]).reshape(8, 64)   # (heads, S)
        heads = slice(hg * 8, hg * 8 + 8)
        ph = np.exp(1j * om[heads] * float(L))              # f64 (8, S)
        h_last[b, heads] = (v * ph).astype(np.complex64)

    if _trace:
        return (y, h_last), res
    return y, h_last


if __name__ == "__main__":
    rng = np.random.default_rng(0)
    demo = {
        "x": rng.standard_normal((BATCH, L, D), dtype=np.float32),
        "log_alpha": np.zeros((H, S_FAST), np.float32),
        "omega": (rng.standard_normal((H, S)) * 0.1).astype(np.float32),
        "B_re": (rng.standard_normal((H, S, DH)) * 0.01).astype(np.float32),
        "B_im": np.zeros((H, S, DH), np.float32),
        "C_re": (rng.standard_normal((H, DH, S)) * 0.01).astype(np.float32),
        "C_im": np.zeros((H, DH, S), np.float32),
        "D_skip": np.full((D,), 0.1, np.float32),
        "W_g": np.zeros((H, S, DH), np.float32),
        "b_g": np.ones((H, S), np.float32),
        "h0": np.zeros((BATCH, H, S), np.complex64),
    }
    y, hl = kernel(**demo)
    print("ran:", y.shape, hl.shape, y.dtype, hl.dtype)
